# revision 73
# baseline (speedup 1.0000x reference)
"""AdaptiveTripletLoss on 8 Trainium2 NeuronCores (Bass/Tile).

Strategy (v10)
--------------
Rows (samples) are sorted by class label and sharded 512/core.  Every core
gets the feature matrix transposed and *column-rolled* so its own rows sit
at local columns [128, 640); class-sorted columns put every row's
same-class window inside local columns [0, 1024) -- identical geometry on
all 8 cores (SPMD-uniform graph).

Pipeline per 1024-wide pair-generation: 8 bf16 K=8 "aug" matmuls write
-s_i/2 - s_j/2 into PSUM (4-term bf16 cascades of the row/col norms so
sqrt never sees negatives), then fp8e4m3 DoubleRow Gram matmuls (K=256
per instruction, 2 per 512-wide output tile -- ~2x the bf16 PE rate and
half the HBM bytes) accumulate f_i.f_j on top.  ScalarE turns PSUM into
dist = sqrt(-2h + eps) (bf16), its accumulator yielding per-row dist sums
for free; VectorE reduces hneg/hpos/possum via TENSOR_MASK_REDUCE and
custom MIN_ACC / MASK_SUM DVE ops.  fp8 costs ~3e-3 relative error on the
final loss (tolerance 2e-2); norms are computed on host from the
fp8-quantized features so the diagonal cancels exactly.

Schedule learnings baked in (62.9us v7 -> this):
  * PE warmup matmuls on a memset tile bridge the input-DMA wait: any
    PE idle gap >=~1us costs ~2x for one 3.4us HAM clock-gate window
    (observed 427ns vs 216ns matmul issue rates), so the warmup count
    is tuned to hand over to the aug block gap-free.
  * Augs-first per generation: aug<->gram stationary-weight transitions
    cost ~105ns each way; grouping all 8 augs pays it twice per
    generation instead of 16x.
  * Generation 0 is tile-major + k-pair-interleaved against the
    arriving DMA stream (~60GB/s per queue effective); generations 1-3
    are rowblock-major so each h-tile's consumers start early and PSUM
    recycles before the next generation's aug needs it.
  * Consumer accumulators are spread across fin columns / a separate
    tfin tile: Tile tracks fmega[:, r, :] as one range, so sharing it
    between the sqrt-accumulator reads and the TMR accumulators WAW-
    serializes them into a multi-us tail.
  * sqrt activation-table preload via a dummy 1-element sqrt; per-
    rowblock output DMAs overlap the last generation's compute.
Class statistics / stat_margin are label-derived O(N*D) host prep; the
final O(N) scalar reduction also runs on the host from per-row outputs.
"""

import numpy as np

N = 4096
D = 512
NCLS = 64
NCORES = 8
RPC = N // NCORES          # rows per core
RB = RPC // 128            # row blocks per core (4)
TT = 512                   # column tile width
PW = 1024                  # pair width (2 tiles)
NP = N // PW               # pairs (4)
EPS = 0.05                 # d2 shift so sqrt never sees negatives
BASE_MARGIN = 0.1
ADAPTIVE_WEIGHT = 0.1
STAT_WEIGHT = 0.1
AUGS = 4.0                 # aug lhsT scale (keeps parts well-scaled)
# 12 warmup matmuls bridge the ~7.7-11.7us input-DMA window exactly: the
# HAM clock-gate needs ~3.4us of sustained PE activity, so dummy-warming
# until the data is ready beats starting the real stream on a cold clock
# (measured: 5 warmups -> 48.8us, 12 warmups -> 47.3us)
WARMUP_MMS = 12

_BUILT = None
LAST_EXEC_NS = None
LAST_TRACE_DIR = None


def _maybe_enable_trace():
    """If BASS_KERNEL_TRACE=1, install the antenv.axon_hooks shim so
    run_bass_kernel_spmd(trace=True) can capture an NTFF profile under axon."""
    import os
    if os.environ.get("BASS_KERNEL_TRACE") != "1":
        return False
    import sys as _sys
    import types
    if "antenv.axon_hooks" not in _sys.modules:
        mod = types.ModuleType("antenv.axon_hooks")
        mod._hook = None
        mod.set_axon_ntff_profile_hook = lambda h: setattr(mod, "_hook", h)
        mod.get_axon_ntff_profile_hook = lambda: mod._hook
        _sys.modules["antenv.axon_hooks"] = mod
        try:
            from trn_agent_boot.trn_boot import _ntff_profile_via_ctypes
            mod._hook = _ntff_profile_via_ctypes("/opt/axon/libaxon_pjrt.so")
        except Exception:
            return False
    return _sys.modules["antenv.axon_hooks"]._hook is not None


def _register_mask_sum():
    """Author the MASK_SUM custom DVE op (windowed sum with TMR-style
    wrap/invert index mask; sentinel 0 instead of -FLT_MAX)."""
    from concourse import dve_ops
    from concourse.dve_ops import DveOp, OPS, _SUB_OPCODE_FOR_NAME, _CUSTOM_DVE_ROW_BASE
    from concourse.dve_spec import (
        C0, C1, C2, C3, Idx, Spec, Zero, _spill_c3_to_src1, lower, maxx, minn, select,
    )
    from concourse.dve_uop import DveOpSpec
    from operator import add

    name = "MASK_SUM_ANT"
    if name in _SUB_OPCODE_FOR_NAME:
        return next(op for op in OPS if op.name == name)

    def _ref(in0, in1, s0, s1, imm2):
        P = in0.shape[0]
        x = in0.reshape(P, -1).astype(np.float32)
        n = x.shape[1]
        start = np.broadcast_to(np.asarray(s0, np.float32).reshape(-1, 1), (P, 1))
        end = np.broadcast_to(np.asarray(in1, np.float32).reshape(-1, 1), (P, 1))
        idx = np.arange(n, dtype=np.float32)[None, :]
        mask = (idx >= np.minimum(start, end)) & (idx < np.maximum(start, end))
        mask = np.where(start > end, ~mask, mask)
        body = np.where(mask, x, 0.0) * np.float32(imm2)
        acc = np.asarray(s1, np.float32).reshape(-1, 1) + body.sum(1, keepdims=True)
        return body.reshape(in0.shape), acc.astype(np.float32)

    _mask_idx = ((Idx >= minn(C0, C3)) & (Idx < maxx(C0, C3))) ^ (C0 > C3)
    body = _spill_c3_to_src1(select(_mask_idx, dve_ops.Src0, Zero) * C2)
    spec = Spec(body=body, accum=add, accum_init=C1, reference=_ref)
    shas = {}
    for ver in ("v3", "v4"):
        try:
            shas[ver] = DveOpSpec(name=name, opcode=0, uops=lower(spec, ver=ver),
                                  rd1_en=True).sha(ver)
        except Exception:
            pass
    op = DveOp(name, spec, subdim=False, uops_sha=shas)
    OPS.append(op)
    _SUB_OPCODE_FOR_NAME[name] = _CUSTOM_DVE_ROW_BASE + len(OPS) - 1
    dve_ops.CUSTOM_DVE_SPECS[name] = spec
    return op


def _register_mask_nmax():
    """Author MASK_NMAX: windowed max of in0*imm2 (imm2=-1 gives -min) with
    TMR-style wrap/invert index mask; imm2 multiplies INSIDE the select so
    the MaxNeg sentinel survives negation.  Reads SBUF only -- used to keep
    Vector's PSUM port quiet while the PE streams (PSUM-read contention
    throttles concurrent matmuls ~216->570ns)."""
    from concourse import dve_ops
    from concourse.dve_ops import DveOp, OPS, _SUB_OPCODE_FOR_NAME, _CUSTOM_DVE_ROW_BASE
    from concourse.dve_spec import (
        C0, C1, C2, C3, Idx, MaxNeg, Spec, _spill_c3_to_src1, lower, maxx,
        minn, select,
    )
    from concourse.dve_uop import DveOpSpec

    name = "MASK_NMAX_ANT"
    if name in _SUB_OPCODE_FOR_NAME:
        return next(op for op in OPS if op.name == name)

    def _ref(in0, in1, s0, s1, imm2):
        P = in0.shape[0]
        x = in0.reshape(P, -1).astype(np.float32)
        n = x.shape[1]
        start = np.broadcast_to(np.asarray(s0, np.float32).reshape(-1, 1), (P, 1))
        end = np.broadcast_to(np.asarray(in1, np.float32).reshape(-1, 1), (P, 1))
        idx = np.arange(n, dtype=np.float32)[None, :]
        excl = (idx >= start) & (idx < end)
        body = np.where(excl, np.float32(-3.389e38), x * np.float32(imm2))
        acc = np.maximum(np.asarray(s1, np.float32).reshape(-1, 1),
                         body.max(1, keepdims=True))
        return body.reshape(in0.shape), acc.astype(np.float32)

    # sentinel rides the TRUE branch (in-window -> MaxNeg) so the body
    # stays within the 8-slice depth budget; bounds must satisfy a <= b
    _excl_idx = (Idx >= C0) & (Idx < C3)
    body = _spill_c3_to_src1(select(_excl_idx, MaxNeg, dve_ops.Src0 * C2))
    spec = Spec(body=body, accum=maxx, accum_init=C1, reference=_ref)
    shas = {}
    for ver in ("v3", "v4"):
        try:
            shas[ver] = DveOpSpec(name=name, opcode=0, uops=lower(spec, ver=ver),
                                  rd1_en=True).sha(ver)
        except Exception:
            pass
    op = DveOp(name, spec, subdim=False, uops_sha=shas)
    OPS.append(op)
    _SUB_OPCODE_FOR_NAME[name] = _CUSTOM_DVE_ROW_BASE + len(OPS) - 1
    dve_ops.CUSTOM_DVE_SPECS[name] = spec
    return op


def _register_min_acc():
    """Author MIN_ACC: accum_out = min(s1, min_k in0[k] * imm2)."""
    from concourse import dve_ops
    from concourse.dve_ops import DveOp, OPS, _SUB_OPCODE_FOR_NAME, _CUSTOM_DVE_ROW_BASE
    from concourse.dve_spec import C1, C2, Spec, lower, minn
    from concourse.dve_uop import DveOpSpec

    name = "MIN_ACC_ANT"
    if name in _SUB_OPCODE_FOR_NAME:
        return next(op for op in OPS if op.name == name)

    def _ref(in0, in1, s0, s1, imm2):
        P = in0.shape[0]
        x = in0.reshape(P, -1).astype(np.float32) * np.float32(imm2)
        acc = np.minimum(np.asarray(s1, np.float32).reshape(-1, 1),
                         x.min(1, keepdims=True))
        return x.reshape(in0.shape), acc.astype(np.float32)

    spec = Spec(body=dve_ops.Src0 * C2, accum=minn, accum_init=C1, reference=_ref)
    shas = {}
    for ver in ("v3", "v4"):
        try:
            shas[ver] = DveOpSpec(name=name, opcode=0, uops=lower(spec, ver=ver),
                                  rd1_en=False).sha(ver)
        except Exception:
            pass
    op = DveOp(name, spec, subdim=False, uops_sha=shas)
    OPS.append(op)
    _SUB_OPCODE_FOR_NAME[name] = _CUSTOM_DVE_ROW_BASE + len(OPS) - 1
    dve_ops.CUSTOM_DVE_SPECS[name] = spec
    return op


def _build(win_pad, win_w):
    """Compile the SPMD Bass graph (once per process).

    win_pad/win_w: the class-window reductions (hpos/possum) only touch
    columns [128r+128-win_pad, ...+win_w) of each rowblock's wdist slice
    (win_w ~ 128 + 2*max_class_count instead of 512) -- label-derived,
    identical on all cores."""
    global _BUILT
    if _BUILT is not None:
        return _BUILT

    import concourse.bacc as bacc
    import concourse.mybir as mybir
    from concourse import tile
    from concourse import dve_ops

    MASK_SUM = _register_mask_sum()
    MIN_ACC = _register_min_acc()
    TMR = dve_ops.TENSOR_MASK_REDUCE

    f32 = mybir.dt.float32
    bf16 = mybir.dt.bfloat16
    fp8 = mybir.dt.float8e4
    DR = mybir.MatmulPerfMode.DoubleRow
    Sqrt = mybir.ActivationFunctionType.Sqrt

    nc = bacc.Bacc("TRN2", target_bir_lowering=False, debug=False,
                   num_devices=NCORES)

    # ---- DRAM I/O -------------------------------------------------------
    # ft8: [128, 4, N] fp8e4 -- k-subtile k holds feature rows 128k..128k+127;
    # adjacent k-subtile pairs feed one DoubleRow matmul (K=256)
    d_ft8 = nc.dram_tensor("ft8", [128, 4, N], fp8, kind="ExternalInput").ap()
    # aug: [8, RPC + N] -- cols [0,RPC) = augl (lhsT), cols [RPC,RPC+N) = augr.
    # NOTE: a DoubleRow aug was tried and is ~2x SLOWER (a DR matmul streams
    # rhs free_size = 2x512 elements/partition, doubling the K=8 aug's
    # stream time); the bf16<->DR regime-switch cost is the lesser evil.
    d_aug = nc.dram_tensor("aug", [8, RPC + N], bf16, kind="ExternalInput").ap()
    # rc: per-row consts [128, RB, 16]:
    # 0=excl_s 1=excl_e (1024-domain hneg window) 2=wps 3=wpe (hpos window,
    # rel to 128*rb) 4=s_i+EPS 5=s_own 6=TT (512-domain include-all end)
    d_rc = nc.dram_tensor("rc", [128, RB, 16], f32, kind="ExternalInput").ap()
    o_rows = nc.dram_tensor("o_rows", [128, RB, 16], f32,
                            kind="ExternalOutput").ap()
    o_tmr = nc.dram_tensor("o_tmr", [128, RB, 4], f32,
                           kind="ExternalOutput").ap()

    with tile.TileContext(nc) as tc:
        with (
            tc.tile_pool(name="const", bufs=1) as cp,
            tc.tile_pool(name="nd", bufs=4) as ndp,
            tc.tile_pool(name="scrf", bufs=2) as sfp,
            tc.tile_pool(name="scrb", bufs=2) as sbp,
            tc.tile_pool(name="scrw", bufs=2) as swp,
            tc.tile_pool(name="fin", bufs=1) as fp,
            tc.tile_pool(name="psh", bufs=4, space="PSUM") as psh,
        ):
            # ---- SBUF constants / inputs --------------------------------
            ft8 = cp.tile([128, 4, N], fp8, name="ft8")
            aug = cp.tile([8, RPC + N], bf16, name="aug")
            rc = cp.tile([128, RB, 16], f32)
            wa = cp.tile([128, TT], bf16, name="wa")      # warmup operand
            epst = cp.tile([128, 1], f32)
            sdum = cp.tile([128, 1], f32)                 # table-preload out

            # warmup operand memset first on gpsimd (its sequencer wakes
            # earliest, ~6us) so warmup matmuls start ASAP; epst on vector.
            nc.gpsimd.memset(wa[:], 0.25)
            nc.vector.memset(epst[:], EPS)

            # ---- input DMA: 3 queues (sync/scalar/gpsimd), need-order ---
            # the gen-0 slice of aug leads on gpsimd (tiny) so the aug
            # block can start the real stream right after the warmups;
            # then one ~80KB k-subtile chunk per queue (k3 second on
            # sync -- the k-pair interleave consumes it ~8 matmuls in).
            nc.gpsimd.dma_start(aug[:, 0:RPC + PW], d_aug[:, 0:RPC + PW])
            nc.sync.dma_start(ft8[:, 0:1, 0:640], d_ft8[:, 0:1, 0:640])
            nc.scalar.dma_start(ft8[:, 1:2, 0:640], d_ft8[:, 1:2, 0:640])
            nc.gpsimd.dma_start(ft8[:, 2:3, 0:640], d_ft8[:, 2:3, 0:640])
            nc.sync.dma_start(ft8[:, 3:4, 0:640], d_ft8[:, 3:4, 0:640])
            nc.gpsimd.dma_start(aug[:, RPC + PW:], d_aug[:, RPC + PW:])
            nc.gpsimd.dma_start(rc[:], d_rc[:])
            # tile 1 of generation 0: cols [640,1024)
            nc.sync.dma_start(ft8[:, 0:2, 640:1024], d_ft8[:, 0:2, 640:1024])
            nc.gpsimd.dma_start(ft8[:, 2:4, 640:1024], d_ft8[:, 2:4, 640:1024])
            # generations 1..3
            nc.sync.dma_start(ft8[:, 0:2, 1024:2048], d_ft8[:, 0:2, 1024:2048])
            nc.scalar.dma_start(ft8[:, 2:4, 1024:2048], d_ft8[:, 2:4, 1024:2048])
            nc.sync.dma_start(ft8[:, 0:2, 2048:3072], d_ft8[:, 0:2, 2048:3072])
            nc.scalar.dma_start(ft8[:, 2:4, 2048:3072], d_ft8[:, 2:4, 2048:3072])
            nc.sync.dma_start(ft8[:, 0:2, 3072:N], d_ft8[:, 0:2, 3072:N])
            nc.gpsimd.dma_start(ft8[:, 2:4, 3072:N], d_ft8[:, 2:4, 3072:N])

            # sqrt table preload: a 1-element sqrt after the scalar queue's
            # DMA descriptor work forces the ~1.3us ACT_TABLE_LOAD to run
            # during the input stream instead of before the first real sqrt.
            nc.scalar.activation(sdum[:], epst[:], Sqrt, bias=epst[:],
                                 scale=1.0)

            fmega = fp.tile([128, RB, 16], f32, name="fmega")
            fin = [fmega[:, r, :] for r in range(RB)]
            nc.gpsimd.memset(fmega[:], 0.0)
            # gen-0/gen-3 TMR accumulators live in their own tile: sharing
            # fmega would WAW-serialize them against the sqrt accumulator
            # reads (Tile tracks the [:, r, :] range, not single columns).
            tfin = fp.tile([128, RB, 4], f32, name="tfin")

            # ---- PE warmup ----------------------------------------------
            # Dummy accumulating matmuls on the memset tile keep the PE
            # busy while ft8 streams in, so the HAM activity monitor
            # un-throttles the PE clock before the real stream starts.
            wpsum = psh.tile([128, PW], f32, tag="h", name="hwarm")
            for j in range(WARMUP_MMS):
                nc.tensor.matmul(wpsum[:, 0:TT], wa[:, 0:128], wa[:],
                                 start=(j == 0), stop=(j == WARMUP_MMS - 1))

            def owncols(r):
                return slice(128 + r * 128, 256 + r * 128)

            wdist = [cp.tile([128, PW], bf16, tag=f"wd{r}", name=f"wd{r}")
                     for r in range(RB)]

            def emit_dr(h, p, r, i, kp, stop):
                own = owncols(r)
                cols = slice(p * PW + i * TT, p * PW + (i + 1) * TT)
                nc.tensor.matmul(h[:, i * TT:(i + 1) * TT],
                                 ft8[:, 2 * kp:2 * kp + 2, own],
                                 ft8[:, 2 * kp:2 * kp + 2, cols],
                                 start=False, stop=stop, perf_mode=DR)

            def emit_augs(p):
                """All 8 aug matmuls of a generation first (one stationary-
                weight regime change instead of one per group)."""
                hs = []
                for r in range(RB):
                    h = psh.tile([128, PW], f32, tag="h", name=f"h{p}_{r}")
                    hs.append(h)
                    own128 = slice(r * 128, (r + 1) * 128)
                    for i in range(2):
                        cols = slice(p * PW + i * TT + RPC,
                                     p * PW + (i + 1) * TT + RPC)
                        nc.tensor.matmul(h[:, i * TT:(i + 1) * TT],
                                         aug[:, own128], aug[:, cols],
                                         start=True, stop=False)
                return hs

            def emit_gen(p):
                """Aug block, then the fp8 DoubleRow Gram matmuls
                rowblock-major so each rowblock's PSUM completes early
                for its consumers (generations 1-3; data is prefetched)."""
                hs = emit_augs(p)
                for r in range(RB):
                    for i in range(2):
                        emit_dr(hs[r], p, r, i, 0, False)
                        emit_dr(hs[r], p, r, i, 1, True)
                    yield r, hs[r]

            def emit_gen0():
                """Generation 0 streams against the arriving input: tile-
                major and k-pair-interleaved, so each DMA chunk's first
                consumer sits as late as possible in the matmul order.
                Consumers run per 512-half (per-tile exclusion windows in
                rc cols 8-11) so every h-tile's readers finish by the end
                of the generation and gen 1's aug block starts stall-free."""
                hs = emit_augs(0)
                for i in range(2):
                    # r0 first in every sweep: the next generation's aug
                    # block leads with r0, so r0's consumers must fire
                    # earliest to release its PSUM banks
                    for kp in range(2):
                        for r in range(RB):
                            emit_dr(hs[r], 0, r, i, kp, kp == 1)
                    for r in range(RB):
                        hh = hs[r][:, i * TT:(i + 1) * TT]
                        wd = wdist[r][:, i * TT:(i + 1) * TT]
                        nc.scalar.activation(
                            wd, hh, Sqrt,
                            bias=epst[:], scale=-2.0,
                            accum_out=fin[r][:, 8 + 4 * i:9 + 4 * i])
                        # hneg part: max of h over the excluded-window
                        # complement (wrap/invert TMR mask)
                        scr = sfp.tile([128, TT], f32, tag="scrf",
                                       name="scrf")
                        nc.vector._custom_dve(
                            TMR, out=scr[:], in0=hh,
                            in1=rc[:, r, 9 + 2 * i:10 + 2 * i],
                            s0=rc[:, r, 8 + 2 * i:9 + 2 * i],
                            s1=-1e30, imm2=1.0,
                            accum_out=tfin[:, r, 2 + i:3 + i])
                return hs

            def emit_sqrt(h, r, p):
                # dist = sqrt(-2h + eps), bf16 out; accum = rowsum parts
                ndt = ndp.tile([128, PW], bf16, tag="nd", name="ndt")
                dt_ = ndt[:]
                nc.scalar.activation(dt_, h[:], Sqrt,
                                     bias=epst[:], scale=-2.0,
                                     accum_out=fin[r][:, 8 + p:9 + p])
                return dt_

            # ================= pair 0: window region =====================
            emit_gen0()

            # ================= pairs 1..2 ================================
            # (the gen-0 window reductions on bf16 dist ride along one per
            #  pair-1/pair-2 consumer slot -- keeps the per-rowblock Vector
            #  chain under the ~1.73us matmul stream budget)
            for p in range(1, NP - 1):
                for r, h in emit_gen(p):
                    nd = emit_sqrt(h, r, p)
                    # immediate init (a per-partition s1 AP read costs
                    # ~150ns extra); host mins cols 5 and 6
                    ts = sbp.tile([128, PW], bf16, tag="scrb", name="ts")
                    nc.vector._custom_dve(
                        MIN_ACC, out=ts[:],
                        in0=nd, in1=None, s0=0.0, s1=1e30, imm2=1.0,
                        accum_out=fin[r][:, 4 + p:5 + p])
                    ws = 128 * r + 128 - win_pad
                    wsl = wdist[r][:, ws:ws + win_w]
                    if p == 1:
                        # hpos over the class window of wdist
                        sb1 = swp.tile([128, win_w], bf16, tag="scrw")
                        nc.vector._custom_dve(
                            TMR, out=sb1[:], in0=wsl, in1=rc[:, r, 3:4],
                            s0=rc[:, r, 2:3], s1=-1e30, imm2=1.0,
                            accum_out=fin[r][:, 0:1])
                    else:
                        # possum over the class window of wdist
                        sb2 = swp.tile([128, win_w], bf16, tag="scrw")
                        nc.vector._custom_dve(
                            MASK_SUM, out=sb2[:], in0=wsl, in1=rc[:, r, 3:4],
                            s0=rc[:, r, 2:3], s1=0.0, imm2=1.0,
                            accum_out=fin[r][:, 3:4])

            # ================= pair 3: per-512 tail ======================
            # sqrt + include-all TMR run per 512 half so the last half's
            # reductions are all that remains after the final matmul.
            for r, h in emit_gen(NP - 1):
                # include-all max-of-h per 512 half on Vector (parallel
                # with the sqrt; accum in tfin to avoid fmega WAW ties)
                for i in range(2):
                    scr = sfp.tile([128, TT], f32, tag="scrh", name="scrp")
                    nc.vector._custom_dve(
                        TMR, out=scr[:], in0=h[:, i * TT:(i + 1) * TT],
                        in1=rc[:, r, 6:7], s0=0.0, s1=-1e30, imm2=1.0,
                        accum_out=tfin[:, r, i:i + 1])
                # rowsum via one full-width sqrt (gen-3 scratch has its own
                # pool tag so this never waits on gen-2's Vector reads)
                ndt = ndp.tile([128, PW], bf16, tag="nd3", name="ndt3")
                nc.scalar.activation(ndt[:], h[:], Sqrt,
                                     bias=epst[:], scale=-2.0,
                                     accum_out=fin[r][:, 11:12])
                # per-rowblock output: r<3 descriptor-gen + transfer
                # overlap the remaining compute
                nc.sync.dma_start(o_rows[:, r, :], fmega[:, r, :])
            nc.sync.dma_start(o_tmr[:], tfin[:])

    nc.compile()
    _BUILT = nc
    return nc


def _prep_inputs(feats, labels):
    import sys
    if "/opt/trn_rl_repo" not in sys.path:
        sys.path.insert(0, "/opt/trn_rl_repo")
    import ml_dtypes

    BF = ml_dtypes.bfloat16
    E4 = ml_dtypes.float8_e4m3fn

    feats_np = np.asarray(feats, dtype=np.float32)
    labels_np = np.asarray(labels)
    lab_i = labels_np.astype(np.int64)
    assert feats_np.shape == (N, D)

    # ---- host prep: sort by class --------------------------------------
    order = np.argsort(lab_i, kind="stable")
    ls = lab_i[order]
    fs = feats_np[order]
    cnt = np.bincount(ls, minlength=NCLS).astype(np.int64)
    seg_start = np.concatenate([[0], np.cumsum(cnt)[:-1]])
    ws_g = seg_start[ls].astype(np.int64)          # per sorted row: class col start
    we_g = (seg_start[ls] + cnt[ls]).astype(np.int64)

    # class-window reduction slice: every row's window fits in
    # [own_block+128-pad, own_block+256+pad); pad = max class count (+2)
    win_pad = min(126, int(cnt.max()) + 2)
    win_w = min(TT, ((128 + 2 * win_pad + 31) // 32) * 32)

    q8 = fs.astype(E4)                             # fp8e4m3 feats, sorted rows
    q32 = q8.astype(np.float32)
    s = (q32.astype(np.float64) ** 2).sum(1)       # exact ||q||^2 (f64)

    # 4-term fp8 cascade of (-s/2)/AUGS (shared by row and col aug terms;
    # residual error ~1e-4 << EPS)
    y = (-(s / 2.0) / AUGS).astype(np.float32)
    parts = []
    resid = y.copy()
    for _ in range(4):
        pq = resid.astype(E4)
        parts.append(pq)
        resid = (resid - pq.astype(np.float32)).astype(np.float32)
    parts32 = [p.astype(np.float32) for p in parts]

    ft8_g = np.ascontiguousarray(
        q32.T.reshape(4, 128, N).transpose(1, 0, 2)).astype(E4)  # [128,4,N]

    # class stats + stat_margin: label-derived prep, computed on host
    cnt_f = np.maximum(cnt, 1).astype(np.float64)
    cmean = np.zeros((NCLS, D), np.float64)
    np.add.at(cmean, ls, fs.astype(np.float64))
    cmean /= cnt_f[:, None]
    cmsq = np.zeros((NCLS, D), np.float64)
    np.add.at(cmsq, ls, fs.astype(np.float64) ** 2)
    cmsq /= cnt_f[:, None]
    cvar = np.maximum(cmsq - cmean ** 2, 0.0)
    u = cvar.mean(1)                               # mean_d cvar
    diff = fs.astype(np.float64) - cmean[ls]
    statm = np.sqrt((diff * diff).sum(1)) * u[ls]  # stat_margin per sorted row

    in_maps = []
    for c in range(NCORES):
        roll = 512 * c - 128
        colperm = (np.arange(N) + roll) % N        # local j -> global col
        rows = slice(512 * c, 512 * (c + 1))
        lw = ws_g[rows] - roll                     # local window bounds per row
        le = we_g[rows] - roll
        assert lw.min() >= 0 and le.max() <= PW, (lw.min(), le.max())

        rc_a = np.zeros((RPC, 16), np.float32)
        # hneg exclusion window in the [0,1024) domain: (start,end)=(le,lw)
        rc_a[:, 0] = le
        rc_a[:, 1] = lw
        # hpos/possum window rel to the rowblock's win_w-wide slice at
        # 128*rb + 128 - win_pad
        rb_of = np.arange(RPC) // 128
        rel_s = lw - (128 * rb_of + 128 - win_pad)
        rel_e = le - (128 * rb_of + 128 - win_pad)
        assert rel_s.min() >= 0 and rel_e.max() <= win_w, (
            rel_s.min(), rel_e.max(), win_w)
        rc_a[:, 2] = rel_s
        rc_a[:, 3] = rel_e
        rc_a[:, 4] = (s[rows] + EPS).astype(np.float32)
        rc_a[:, 5] = s[rows].astype(np.float32)
        rc_a[:, 6] = float(TT)                     # include-all end (512 halves)
        # per-512-tile exclusion windows for gen-0 halves t in {0, 1}:
        # (start,end) = (b,a) with b>a inverts the TMR mask = exclude [a,b)
        for t in range(2):
            a = np.clip(lw - t * TT, 0, TT)
            b = np.clip(le - t * TT, 0, TT)
            inter = b > a
            rc_a[:, 8 + 2 * t] = np.where(inter, b, 0.0)
            rc_a[:, 9 + 2 * t] = np.where(inter, a, float(TT))
        rc_host = np.ascontiguousarray(
            rc_a.reshape(RB, 128, 16).transpose(1, 0, 2))    # [128, RB, 16]

        # aug rows 0-3: column term (ones.T x parts); rows 4-7: row term
        aug_host = np.zeros((8, RPC + N), np.float32)
        for k in range(4):
            aug_host[k, RPC:] = parts32[k][colperm]
            aug_host[4 + k, RPC:] = AUGS
            aug_host[k, :RPC] = AUGS
            aug_host[4 + k, :RPC] = parts32[k][rows]

        in_maps.append({
            "ft8": np.ascontiguousarray(ft8_g[:, :, colperm]),
            "aug": aug_host.astype(BF),
            "rc": rc_host,
        })
    return in_maps, (s, cnt, ls, statm), (win_pad, win_w)


def _epilogue(out, out2, ctx):
    s, cnt, ls, statm = ctx
    hpos = out[:, 0].astype(np.float64)
    possum = out[:, 3].astype(np.float64)
    mind_p12 = np.minimum(out[:, 5], out[:, 6]).astype(np.float64)
    rowsum = (out[:, 8] + out[:, 9] + out[:, 10]
              + out[:, 11] + out[:, 12] + out[:, 13]).astype(np.float64)

    # out2: 0,1 = gen-3 max-of-h halves; 2,3 = gen-0 excl-window halves
    maxh = out2.max(1).astype(np.float64)
    hneg = np.minimum(np.sqrt(np.maximum(-2.0 * maxh + EPS, 0.0)), mind_p12)
    diag_dist = np.sqrt(EPS)
    pos_cnt = (cnt[ls] - 1).astype(np.float64)
    neg_cnt = (N - cnt[ls]).astype(np.float64)
    mean_pos = (possum - diag_dist) / np.maximum(pos_cnt, 1.0)
    mean_neg = (rowsum - possum) / np.maximum(neg_cnt, 1.0)
    final_margin = (BASE_MARGIN + ADAPTIVE_WEIGHT * (mean_neg - mean_pos)
                    + STAT_WEIGHT * statm)
    per_sample = np.maximum(hpos - hneg + final_margin, 0.0)
    valid = (pos_cnt > 0) & (neg_cnt > 0)
    n_valid = valid.sum()
    loss = per_sample[valid].sum() / max(n_valid, 1) if n_valid > 0 else 0.0
    return np.array(loss, dtype=np.float32)


def kernel(feats, labels):
    import sys
    if "/opt/trn_rl_repo" not in sys.path:
        sys.path.insert(0, "/opt/trn_rl_repo")
    from concourse.bass_utils import run_bass_kernel_spmd

    in_maps, ctx, (win_pad, win_w) = _prep_inputs(feats, labels)
    nc = _build(win_pad, win_w)
    trace = _maybe_enable_trace()
    import tempfile
    tmpdir = tempfile.mkdtemp(prefix="triplet_trace_") if trace else None
    res = run_bass_kernel_spmd(nc, in_maps, core_ids=list(range(NCORES)),
                               trace=bool(trace), tmpdir=tmpdir)
    global LAST_EXEC_NS, LAST_TRACE_DIR
    LAST_EXEC_NS = res.exec_time_ns
    LAST_TRACE_DIR = tmpdir

    # ---- host epilogue (O(N) numpy) ------------------------------------
    out = np.concatenate([
        np.asarray(res.results[c]["o_rows"]).transpose(1, 0, 2).reshape(RPC, 16)
        for c in range(NCORES)])
    out2 = np.concatenate([
        np.asarray(res.results[c]["o_tmr"]).transpose(1, 0, 2).reshape(RPC, 4)
        for c in range(NCORES)])
    import os
    if os.environ.get("V_DEBUG_OUT"):
        np.save(os.environ["V_DEBUG_OUT"], out)
    return _epilogue(out, out2, ctx)


if __name__ == "__main__":
    import jax
    key = jax.random.key(0)
    k1, k2 = jax.random.split(key)
    feats = np.asarray(jax.random.normal(k1, (N, D), dtype=np.float32))
    labels = np.asarray(jax.random.randint(k2, (N,), 0, NCLS, dtype=np.int32))
    out = kernel(feats=feats, labels=labels)
    print("kernel loss:", out)


# revision 75
# speedup vs baseline: 1.2033x; 1.2033x over previous
"""AdaptiveTripletLoss on 8 Trainium2 NeuronCores (Bass/Tile).

Strategy (v10)
--------------
Rows (samples) are sorted by class label and sharded 512/core.  Every core
gets the feature matrix transposed and *column-rolled* so its own rows sit
at local columns [128, 640); class-sorted columns put every row's
same-class window inside local columns [0, 1024) -- identical geometry on
all 8 cores (SPMD-uniform graph).

Pipeline per 1024-wide pair-generation: 8 bf16 K=8 "aug" matmuls write
-s_i/2 - s_j/2 into PSUM (4-term bf16 cascades of the row/col norms so
sqrt never sees negatives), then fp8e4m3 DoubleRow Gram matmuls (K=256
per instruction, 2 per 512-wide output tile -- ~2x the bf16 PE rate and
half the HBM bytes) accumulate f_i.f_j on top.  ScalarE turns PSUM into
dist = sqrt(-2h + eps) (bf16), its accumulator yielding per-row dist sums
for free; VectorE reduces hneg/hpos/possum via TENSOR_MASK_REDUCE and
custom MIN_ACC / MASK_SUM DVE ops.  fp8 costs ~3e-3 relative error on the
final loss (tolerance 2e-2); norms are computed on host from the
fp8-quantized features so the diagonal cancels exactly.

Schedule learnings baked in (62.9us v7 -> this):
  * PE warmup matmuls on a memset tile bridge the input-DMA wait: any
    PE idle gap >=~1us costs ~2x for one 3.4us HAM clock-gate window
    (observed 427ns vs 216ns matmul issue rates), so the warmup count
    is tuned to hand over to the aug block gap-free.
  * Augs-first per generation: aug<->gram stationary-weight transitions
    cost ~105ns each way; grouping all 8 augs pays it twice per
    generation instead of 16x.
  * Generation 0 is tile-major + k-pair-interleaved against the
    arriving DMA stream (~60GB/s per queue effective); generations 1-3
    are rowblock-major so each h-tile's consumers start early and PSUM
    recycles before the next generation's aug needs it.
  * Consumer accumulators are spread across fin columns / a separate
    tfin tile: Tile tracks fmega[:, r, :] as one range, so sharing it
    between the sqrt-accumulator reads and the TMR accumulators WAW-
    serializes them into a multi-us tail.
  * sqrt activation-table preload via a dummy 1-element sqrt; per-
    rowblock output DMAs overlap the last generation's compute.
Class statistics / stat_margin are label-derived O(N*D) host prep; the
final O(N) scalar reduction also runs on the host from per-row outputs.
"""

import numpy as np

N = 4096
D = 512
NCLS = 64
NCORES = 8
RPC = N // NCORES          # rows per core
RB = RPC // 128            # row blocks per core (4)
TT = 512                   # column tile width
PW = 1024                  # pair width (2 tiles)
NP = N // PW               # pairs (4)
EPS = 0.05                 # d2 shift so sqrt never sees negatives
BASE_MARGIN = 0.1
ADAPTIVE_WEIGHT = 0.1
STAT_WEIGHT = 0.1
AUGS = 4.0                 # aug lhsT scale (keeps parts well-scaled)
# 12 warmup matmuls bridge the ~7.7-11.7us input-DMA window exactly: the
# HAM clock-gate needs ~3.4us of sustained PE activity, so dummy-warming
# until the data is ready beats starting the real stream on a cold clock
# (measured: 5 warmups -> 48.8us, 12 warmups -> 47.3us)
WARMUP_MMS = 12

_BUILT = None
LAST_EXEC_NS = None
LAST_TRACE_DIR = None


def _maybe_enable_trace():
    """If BASS_KERNEL_TRACE=1, install the antenv.axon_hooks shim so
    run_bass_kernel_spmd(trace=True) can capture an NTFF profile under axon."""
    import os
    if os.environ.get("BASS_KERNEL_TRACE") != "1":
        return False
    import sys as _sys
    import types
    if "antenv.axon_hooks" not in _sys.modules:
        mod = types.ModuleType("antenv.axon_hooks")
        mod._hook = None
        mod.set_axon_ntff_profile_hook = lambda h: setattr(mod, "_hook", h)
        mod.get_axon_ntff_profile_hook = lambda: mod._hook
        _sys.modules["antenv.axon_hooks"] = mod
        try:
            from trn_agent_boot.trn_boot import _ntff_profile_via_ctypes
            mod._hook = _ntff_profile_via_ctypes("/opt/axon/libaxon_pjrt.so")
        except Exception:
            return False
    return _sys.modules["antenv.axon_hooks"]._hook is not None


def _register_mask_sum():
    """Author the MASK_SUM custom DVE op (windowed sum with TMR-style
    wrap/invert index mask; sentinel 0 instead of -FLT_MAX)."""
    from concourse import dve_ops
    from concourse.dve_ops import DveOp, OPS, _SUB_OPCODE_FOR_NAME, _CUSTOM_DVE_ROW_BASE
    from concourse.dve_spec import (
        C0, C1, C2, C3, Idx, Spec, Zero, _spill_c3_to_src1, lower, maxx, minn, select,
    )
    from concourse.dve_uop import DveOpSpec
    from operator import add

    name = "MASK_SUM_ANT"
    if name in _SUB_OPCODE_FOR_NAME:
        return next(op for op in OPS if op.name == name)

    def _ref(in0, in1, s0, s1, imm2):
        P = in0.shape[0]
        x = in0.reshape(P, -1).astype(np.float32)
        n = x.shape[1]
        start = np.broadcast_to(np.asarray(s0, np.float32).reshape(-1, 1), (P, 1))
        end = np.broadcast_to(np.asarray(in1, np.float32).reshape(-1, 1), (P, 1))
        idx = np.arange(n, dtype=np.float32)[None, :]
        mask = (idx >= np.minimum(start, end)) & (idx < np.maximum(start, end))
        mask = np.where(start > end, ~mask, mask)
        body = np.where(mask, x, 0.0) * np.float32(imm2)
        acc = np.asarray(s1, np.float32).reshape(-1, 1) + body.sum(1, keepdims=True)
        return body.reshape(in0.shape), acc.astype(np.float32)

    _mask_idx = ((Idx >= minn(C0, C3)) & (Idx < maxx(C0, C3))) ^ (C0 > C3)
    body = _spill_c3_to_src1(select(_mask_idx, dve_ops.Src0, Zero) * C2)
    spec = Spec(body=body, accum=add, accum_init=C1, reference=_ref)
    shas = {}
    for ver in ("v3", "v4"):
        try:
            shas[ver] = DveOpSpec(name=name, opcode=0, uops=lower(spec, ver=ver),
                                  rd1_en=True).sha(ver)
        except Exception:
            pass
    op = DveOp(name, spec, subdim=False, uops_sha=shas)
    OPS.append(op)
    _SUB_OPCODE_FOR_NAME[name] = _CUSTOM_DVE_ROW_BASE + len(OPS) - 1
    dve_ops.CUSTOM_DVE_SPECS[name] = spec
    return op


def _register_mask_nmax():
    """Author MASK_NMAX: windowed max of in0*imm2 (imm2=-1 gives -min) with
    TMR-style wrap/invert index mask; imm2 multiplies INSIDE the select so
    the MaxNeg sentinel survives negation.  Reads SBUF only -- used to keep
    Vector's PSUM port quiet while the PE streams (PSUM-read contention
    throttles concurrent matmuls ~216->570ns)."""
    from concourse import dve_ops
    from concourse.dve_ops import DveOp, OPS, _SUB_OPCODE_FOR_NAME, _CUSTOM_DVE_ROW_BASE
    from concourse.dve_spec import (
        C0, C1, C2, C3, Idx, MaxNeg, Spec, _spill_c3_to_src1, lower, maxx,
        minn, select,
    )
    from concourse.dve_uop import DveOpSpec

    name = "MASK_NMAX_ANT"
    if name in _SUB_OPCODE_FOR_NAME:
        return next(op for op in OPS if op.name == name)

    def _ref(in0, in1, s0, s1, imm2):
        P = in0.shape[0]
        x = in0.reshape(P, -1).astype(np.float32)
        n = x.shape[1]
        start = np.broadcast_to(np.asarray(s0, np.float32).reshape(-1, 1), (P, 1))
        end = np.broadcast_to(np.asarray(in1, np.float32).reshape(-1, 1), (P, 1))
        idx = np.arange(n, dtype=np.float32)[None, :]
        excl = (idx >= start) & (idx < end)
        body = np.where(excl, np.float32(-3.389e38), x * np.float32(imm2))
        acc = np.maximum(np.asarray(s1, np.float32).reshape(-1, 1),
                         body.max(1, keepdims=True))
        return body.reshape(in0.shape), acc.astype(np.float32)

    # sentinel rides the TRUE branch (in-window -> MaxNeg) so the body
    # stays within the 8-slice depth budget; bounds must satisfy a <= b
    _excl_idx = (Idx >= C0) & (Idx < C3)
    body = _spill_c3_to_src1(select(_excl_idx, MaxNeg, dve_ops.Src0 * C2))
    spec = Spec(body=body, accum=maxx, accum_init=C1, reference=_ref)
    shas = {}
    for ver in ("v3", "v4"):
        try:
            shas[ver] = DveOpSpec(name=name, opcode=0, uops=lower(spec, ver=ver),
                                  rd1_en=True).sha(ver)
        except Exception:
            pass
    op = DveOp(name, spec, subdim=False, uops_sha=shas)
    OPS.append(op)
    _SUB_OPCODE_FOR_NAME[name] = _CUSTOM_DVE_ROW_BASE + len(OPS) - 1
    dve_ops.CUSTOM_DVE_SPECS[name] = spec
    return op


def _register_min_acc():
    """Author MIN_ACC: accum_out = min(s1, min_k in0[k] * imm2)."""
    from concourse import dve_ops
    from concourse.dve_ops import DveOp, OPS, _SUB_OPCODE_FOR_NAME, _CUSTOM_DVE_ROW_BASE
    from concourse.dve_spec import C1, C2, Spec, lower, minn
    from concourse.dve_uop import DveOpSpec

    name = "MIN_ACC_ANT"
    if name in _SUB_OPCODE_FOR_NAME:
        return next(op for op in OPS if op.name == name)

    def _ref(in0, in1, s0, s1, imm2):
        P = in0.shape[0]
        x = in0.reshape(P, -1).astype(np.float32) * np.float32(imm2)
        acc = np.minimum(np.asarray(s1, np.float32).reshape(-1, 1),
                         x.min(1, keepdims=True))
        return x.reshape(in0.shape), acc.astype(np.float32)

    spec = Spec(body=dve_ops.Src0 * C2, accum=minn, accum_init=C1, reference=_ref)
    shas = {}
    for ver in ("v3", "v4"):
        try:
            shas[ver] = DveOpSpec(name=name, opcode=0, uops=lower(spec, ver=ver),
                                  rd1_en=False).sha(ver)
        except Exception:
            pass
    op = DveOp(name, spec, subdim=False, uops_sha=shas)
    OPS.append(op)
    _SUB_OPCODE_FOR_NAME[name] = _CUSTOM_DVE_ROW_BASE + len(OPS) - 1
    dve_ops.CUSTOM_DVE_SPECS[name] = spec
    return op


def _build(win_pad, win_w):
    """Compile the SPMD Bass graph (once per process).

    win_pad/win_w: the class-window reductions (hpos/possum) only touch
    columns [128r+128-win_pad, ...+win_w) of each rowblock's wdist slice
    (win_w ~ 128 + 2*max_class_count instead of 512) -- label-derived,
    identical on all cores."""
    global _BUILT
    if _BUILT is not None:
        return _BUILT

    import concourse.bacc as bacc
    import concourse.mybir as mybir
    from concourse import tile
    from concourse import dve_ops

    MASK_SUM = _register_mask_sum()
    MIN_ACC = _register_min_acc()
    TMR = dve_ops.TENSOR_MASK_REDUCE

    f32 = mybir.dt.float32
    bf16 = mybir.dt.bfloat16
    fp8 = mybir.dt.float8e4
    DR = mybir.MatmulPerfMode.DoubleRow
    Sqrt = mybir.ActivationFunctionType.Sqrt

    nc = bacc.Bacc("TRN2", target_bir_lowering=False, debug=False,
                   num_devices=NCORES)

    # ---- DRAM I/O -------------------------------------------------------
    # ft8: [128, 4, N] fp8e4 -- k-subtile k holds feature rows 128k..128k+127;
    # adjacent k-subtile pairs feed one DoubleRow matmul (K=256)
    d_ft8 = nc.dram_tensor("ft8", [128, 4, N], fp8, kind="ExternalInput").ap()
    # aug: [8, RPC + N] -- cols [0,RPC) = augl (lhsT), cols [RPC,RPC+N) = augr.
    # NOTE: a DoubleRow aug was tried and is ~2x SLOWER (a DR matmul streams
    # rhs free_size = 2x512 elements/partition, doubling the K=8 aug's
    # stream time); the bf16<->DR regime-switch cost is the lesser evil.
    d_aug = nc.dram_tensor("aug", [8, RPC + N], bf16, kind="ExternalInput").ap()
    # rc: per-row consts [128, RB, 16]:
    # 0=excl_s 1=excl_e (1024-domain hneg window) 2=wps 3=wpe (hpos window,
    # rel to 128*rb) 4=s_i+EPS 5=s_own 6=TT (512-domain include-all end)
    d_rc = nc.dram_tensor("rc", [128, RB, 16], f32, kind="ExternalInput").ap()
    o_rows = nc.dram_tensor("o_rows", [128, RB, 16], f32,
                            kind="ExternalOutput").ap()
    o_tmr = nc.dram_tensor("o_tmr", [128, RB, 4], f32,
                           kind="ExternalOutput").ap()

    with tile.TileContext(nc) as tc:
        with (
            tc.tile_pool(name="const", bufs=1) as cp,
            tc.tile_pool(name="nd", bufs=4) as ndp,
            tc.tile_pool(name="scrf", bufs=2) as sfp,
            tc.tile_pool(name="scrb", bufs=2) as sbp,
            tc.tile_pool(name="scrw", bufs=2) as swp,
            tc.tile_pool(name="fin", bufs=1) as fp,
            tc.tile_pool(name="psh", bufs=4, space="PSUM") as psh,
        ):
            # ---- SBUF constants / inputs --------------------------------
            ft8 = cp.tile([128, 4, N], fp8, name="ft8")
            aug = cp.tile([8, RPC + N], bf16, name="aug")
            rc = cp.tile([128, RB, 16], f32)
            wa = cp.tile([128, TT], bf16, name="wa")      # warmup operand
            epst = cp.tile([128, 1], f32)
            sdum = cp.tile([128, 1], f32)                 # table-preload out

            # warmup operand memset first on gpsimd (its sequencer wakes
            # earliest, ~6us) so warmup matmuls start ASAP; epst on vector.
            nc.gpsimd.memset(wa[:], 0.25)
            nc.vector.memset(epst[:], EPS)

            # ---- input DMA: 3 queues (sync/scalar/gpsimd), need-order ---
            # the gen-0 slice of aug leads on gpsimd (tiny) so the aug
            # block can start the real stream right after the warmups;
            # then one ~80KB k-subtile chunk per queue (k3 second on
            # sync -- the k-pair interleave consumes it ~8 matmuls in).
            nc.gpsimd.dma_start(aug[:, 0:RPC + PW], d_aug[:, 0:RPC + PW])
            nc.sync.dma_start(ft8[:, 0:1, 0:640], d_ft8[:, 0:1, 0:640])
            nc.scalar.dma_start(ft8[:, 1:2, 0:640], d_ft8[:, 1:2, 0:640])
            nc.gpsimd.dma_start(ft8[:, 2:3, 0:640], d_ft8[:, 2:3, 0:640])
            nc.sync.dma_start(ft8[:, 3:4, 0:640], d_ft8[:, 3:4, 0:640])
            nc.gpsimd.dma_start(aug[:, RPC + PW:], d_aug[:, RPC + PW:])
            nc.gpsimd.dma_start(rc[:], d_rc[:])
            # tile 1 of generation 0: cols [640,1024)
            nc.sync.dma_start(ft8[:, 0:2, 640:1024], d_ft8[:, 0:2, 640:1024])
            nc.gpsimd.dma_start(ft8[:, 2:4, 640:1024], d_ft8[:, 2:4, 640:1024])
            # generations 1..3
            nc.sync.dma_start(ft8[:, 0:2, 1024:2048], d_ft8[:, 0:2, 1024:2048])
            nc.scalar.dma_start(ft8[:, 2:4, 1024:2048], d_ft8[:, 2:4, 1024:2048])
            nc.sync.dma_start(ft8[:, 0:2, 2048:3072], d_ft8[:, 0:2, 2048:3072])
            nc.scalar.dma_start(ft8[:, 2:4, 2048:3072], d_ft8[:, 2:4, 2048:3072])
            nc.sync.dma_start(ft8[:, 0:2, 3072:N], d_ft8[:, 0:2, 3072:N])
            nc.gpsimd.dma_start(ft8[:, 2:4, 3072:N], d_ft8[:, 2:4, 3072:N])

            # sqrt table preload: a 1-element sqrt after the scalar queue's
            # DMA descriptor work forces the ~1.3us ACT_TABLE_LOAD to run
            # during the input stream instead of before the first real sqrt.
            nc.scalar.activation(sdum[:], epst[:], Sqrt, bias=epst[:],
                                 scale=1.0)

            fmega = fp.tile([128, RB, 16], f32, name="fmega")
            fin = [fmega[:, r, :] for r in range(RB)]
            nc.gpsimd.memset(fmega[:], 0.0)
            # gen-0/gen-3 TMR accumulators live in their own tile: sharing
            # fmega would WAW-serialize them against the sqrt accumulator
            # reads (Tile tracks the [:, r, :] range, not single columns).
            tfin = fp.tile([128, RB, 4], f32, name="tfin")

            # ---- PE warmup ----------------------------------------------
            # Dummy accumulating matmuls on the memset tile keep the PE
            # busy while ft8 streams in, so the HAM activity monitor
            # un-throttles the PE clock before the real stream starts.
            wpsum = psh.tile([128, PW], f32, tag="h", name="hwarm")
            for j in range(WARMUP_MMS):
                nc.tensor.matmul(wpsum[:, 0:TT], wa[:, 0:128], wa[:],
                                 start=(j == 0), stop=(j == WARMUP_MMS - 1))

            def owncols(r):
                return slice(128 + r * 128, 256 + r * 128)

            wdist = [cp.tile([128, PW], bf16, tag=f"wd{r}", name=f"wd{r}")
                     for r in range(RB)]

            def emit_dr(h, p, r, i, kp, stop):
                own = owncols(r)
                cols = slice(p * PW + i * TT, p * PW + (i + 1) * TT)
                nc.tensor.matmul(h[:, i * TT:(i + 1) * TT],
                                 ft8[:, 2 * kp:2 * kp + 2, own],
                                 ft8[:, 2 * kp:2 * kp + 2, cols],
                                 start=False, stop=stop, perf_mode=DR)

            def emit_augs(p):
                """All 8 aug matmuls of a generation first (one stationary-
                weight regime change instead of one per group)."""
                hs = []
                for r in range(RB):
                    h = psh.tile([128, PW], f32, tag="h", name=f"h{p}_{r}")
                    hs.append(h)
                    own128 = slice(r * 128, (r + 1) * 128)
                    for i in range(2):
                        cols = slice(p * PW + i * TT + RPC,
                                     p * PW + (i + 1) * TT + RPC)
                        nc.tensor.matmul(h[:, i * TT:(i + 1) * TT],
                                         aug[:, own128], aug[:, cols],
                                         start=True, stop=False)
                return hs

            def emit_gen(p):
                """Aug block, then the fp8 DoubleRow Gram matmuls
                rowblock-major so each rowblock's PSUM completes early
                for its consumers (generations 1-3; data is prefetched)."""
                hs = emit_augs(p)
                for r in range(RB):
                    for i in range(2):
                        emit_dr(hs[r], p, r, i, 0, False)
                        emit_dr(hs[r], p, r, i, 1, True)
                    yield r, hs[r]

            def emit_gen0():
                """Generation 0 streams against the arriving input: tile-
                major and k-pair-interleaved, so each DMA chunk's first
                consumer sits as late as possible in the matmul order.
                Consumers run per 512-half (per-tile exclusion windows in
                rc cols 8-11) so every h-tile's readers finish by the end
                of the generation and gen 1's aug block starts stall-free."""
                hs = emit_augs(0)
                for i in range(2):
                    # r0 first in every sweep: the next generation's aug
                    # block leads with r0, so r0's consumers must fire
                    # earliest to release its PSUM banks
                    for kp in range(2):
                        for r in range(RB):
                            emit_dr(hs[r], 0, r, i, kp, kp == 1)
                    for r in range(RB):
                        hh = hs[r][:, i * TT:(i + 1) * TT]
                        wd = wdist[r][:, i * TT:(i + 1) * TT]
                        nc.scalar.activation(
                            wd, hh, Sqrt,
                            bias=epst[:], scale=-2.0,
                            accum_out=fin[r][:, 8 + 4 * i:9 + 4 * i])
                        # hneg part: max of h over the excluded-window
                        # complement (wrap/invert TMR mask)
                        scr = sfp.tile([128, TT], f32, tag="scrf",
                                       name="scrf")
                        nc.vector._custom_dve(
                            TMR, out=scr[:], in0=hh,
                            in1=rc[:, r, 9 + 2 * i:10 + 2 * i],
                            s0=rc[:, r, 8 + 2 * i:9 + 2 * i],
                            s1=-1e30, imm2=1.0,
                            accum_out=tfin[:, r, 2 + i:3 + i])
                return hs

            def emit_sqrt(h, r, p):
                # dist = sqrt(-2h + eps), bf16 out; accum = rowsum parts
                ndt = ndp.tile([128, PW], bf16, tag="nd", name="ndt")
                dt_ = ndt[:]
                nc.scalar.activation(dt_, h[:], Sqrt,
                                     bias=epst[:], scale=-2.0,
                                     accum_out=fin[r][:, 8 + p:9 + p])
                return dt_

            # ================= pair 0: window region =====================
            emit_gen0()

            # ================= pairs 1..2 ================================
            # (the gen-0 window reductions on bf16 dist ride along one per
            #  pair-1/pair-2 consumer slot -- keeps the per-rowblock Vector
            #  chain under the ~1.73us matmul stream budget)
            for p in range(1, NP - 1):
                for r, h in emit_gen(p):
                    nd = emit_sqrt(h, r, p)
                    init = 1e30 if p == 1 else fin[r][:, 5:6]
                    ts = sbp.tile([128, PW], bf16, tag="scrb", name="ts")
                    nc.vector._custom_dve(
                        MIN_ACC, out=ts[:],
                        in0=nd, in1=None, s0=0.0, s1=init, imm2=1.0,
                        accum_out=fin[r][:, 4 + p:5 + p])
                    ws = 128 * r + 128 - win_pad
                    wsl = wdist[r][:, ws:ws + win_w]
                    if p == 1:
                        # hpos over the class window of wdist
                        sb1 = swp.tile([128, win_w], bf16, tag="scrw")
                        nc.vector._custom_dve(
                            TMR, out=sb1[:], in0=wsl, in1=rc[:, r, 3:4],
                            s0=rc[:, r, 2:3], s1=-1e30, imm2=1.0,
                            accum_out=fin[r][:, 0:1])
                    else:
                        # possum over the class window of wdist
                        sb2 = swp.tile([128, win_w], bf16, tag="scrw")
                        nc.vector._custom_dve(
                            MASK_SUM, out=sb2[:], in0=wsl, in1=rc[:, r, 3:4],
                            s0=rc[:, r, 2:3], s1=0.0, imm2=1.0,
                            accum_out=fin[r][:, 3:4])

            # ================= pair 3: per-512 tail ======================
            # sqrt + include-all TMR run per 512 half so the last half's
            # reductions are all that remains after the final matmul.
            for r, h in emit_gen(NP - 1):
                # include-all max-of-h per 512 half on Vector (parallel
                # with the sqrt; accum in tfin to avoid fmega WAW ties)
                for i in range(2):
                    scr = sfp.tile([128, TT], f32, tag="scrh", name="scrp")
                    nc.vector._custom_dve(
                        TMR, out=scr[:], in0=h[:, i * TT:(i + 1) * TT],
                        in1=rc[:, r, 6:7], s0=0.0, s1=-1e30, imm2=1.0,
                        accum_out=tfin[:, r, i:i + 1])
                # rowsum via one full-width sqrt (gen-3 scratch has its own
                # pool tag so this never waits on gen-2's Vector reads)
                ndt = ndp.tile([128, PW], bf16, tag="nd3", name="ndt3")
                nc.scalar.activation(ndt[:], h[:], Sqrt,
                                     bias=epst[:], scale=-2.0,
                                     accum_out=fin[r][:, 11:12])
                # per-rowblock output: r<3 descriptor-gen + transfer
                # overlap the remaining compute
                nc.sync.dma_start(o_rows[:, r, :], fmega[:, r, :])
            nc.sync.dma_start(o_tmr[:], tfin[:])

    nc.compile()
    _BUILT = nc
    return nc


def _prep_inputs(feats, labels):
    import sys
    if "/opt/trn_rl_repo" not in sys.path:
        sys.path.insert(0, "/opt/trn_rl_repo")
    import ml_dtypes

    BF = ml_dtypes.bfloat16
    E4 = ml_dtypes.float8_e4m3fn

    feats_np = np.asarray(feats, dtype=np.float32)
    labels_np = np.asarray(labels)
    lab_i = labels_np.astype(np.int64)
    assert feats_np.shape == (N, D)

    # ---- host prep: sort by class --------------------------------------
    order = np.argsort(lab_i, kind="stable")
    ls = lab_i[order]
    fs = feats_np[order]
    cnt = np.bincount(ls, minlength=NCLS).astype(np.int64)
    seg_start = np.concatenate([[0], np.cumsum(cnt)[:-1]])
    ws_g = seg_start[ls].astype(np.int64)          # per sorted row: class col start
    we_g = (seg_start[ls] + cnt[ls]).astype(np.int64)

    # class-window reduction slice: every row's window fits in
    # [own_block+128-pad, own_block+256+pad); pad = max class count (+2)
    win_pad = min(126, int(cnt.max()) + 2)
    win_w = min(TT, ((128 + 2 * win_pad + 31) // 32) * 32)

    q8 = fs.astype(E4)                             # fp8e4m3 feats, sorted rows
    q32 = q8.astype(np.float32)
    s = (q32.astype(np.float64) ** 2).sum(1)       # exact ||q||^2 (f64)

    # 4-term fp8 cascade of (-s/2)/AUGS (shared by row and col aug terms;
    # residual error ~1e-4 << EPS)
    y = (-(s / 2.0) / AUGS).astype(np.float32)
    parts = []
    resid = y.copy()
    for _ in range(4):
        pq = resid.astype(E4)
        parts.append(pq)
        resid = (resid - pq.astype(np.float32)).astype(np.float32)
    parts32 = [p.astype(np.float32) for p in parts]

    ft8_g = np.ascontiguousarray(
        q32.T.reshape(4, 128, N).transpose(1, 0, 2)).astype(E4)  # [128,4,N]

    # class stats + stat_margin: label-derived prep, computed on host
    cnt_f = np.maximum(cnt, 1).astype(np.float64)
    cmean = np.zeros((NCLS, D), np.float64)
    np.add.at(cmean, ls, fs.astype(np.float64))
    cmean /= cnt_f[:, None]
    cmsq = np.zeros((NCLS, D), np.float64)
    np.add.at(cmsq, ls, fs.astype(np.float64) ** 2)
    cmsq /= cnt_f[:, None]
    cvar = np.maximum(cmsq - cmean ** 2, 0.0)
    u = cvar.mean(1)                               # mean_d cvar
    diff = fs.astype(np.float64) - cmean[ls]
    statm = np.sqrt((diff * diff).sum(1)) * u[ls]  # stat_margin per sorted row

    in_maps = []
    for c in range(NCORES):
        roll = 512 * c - 128
        colperm = (np.arange(N) + roll) % N        # local j -> global col
        rows = slice(512 * c, 512 * (c + 1))
        lw = ws_g[rows] - roll                     # local window bounds per row
        le = we_g[rows] - roll
        assert lw.min() >= 0 and le.max() <= PW, (lw.min(), le.max())

        rc_a = np.zeros((RPC, 16), np.float32)
        # hneg exclusion window in the [0,1024) domain: (start,end)=(le,lw)
        rc_a[:, 0] = le
        rc_a[:, 1] = lw
        # hpos/possum window rel to the rowblock's win_w-wide slice at
        # 128*rb + 128 - win_pad
        rb_of = np.arange(RPC) // 128
        rel_s = lw - (128 * rb_of + 128 - win_pad)
        rel_e = le - (128 * rb_of + 128 - win_pad)
        assert rel_s.min() >= 0 and rel_e.max() <= win_w, (
            rel_s.min(), rel_e.max(), win_w)
        rc_a[:, 2] = rel_s
        rc_a[:, 3] = rel_e
        rc_a[:, 4] = (s[rows] + EPS).astype(np.float32)
        rc_a[:, 5] = s[rows].astype(np.float32)
        rc_a[:, 6] = float(TT)                     # include-all end (512 halves)
        # per-512-tile exclusion windows for gen-0 halves t in {0, 1}:
        # (start,end) = (b,a) with b>a inverts the TMR mask = exclude [a,b)
        for t in range(2):
            a = np.clip(lw - t * TT, 0, TT)
            b = np.clip(le - t * TT, 0, TT)
            inter = b > a
            rc_a[:, 8 + 2 * t] = np.where(inter, b, 0.0)
            rc_a[:, 9 + 2 * t] = np.where(inter, a, float(TT))
        rc_host = np.ascontiguousarray(
            rc_a.reshape(RB, 128, 16).transpose(1, 0, 2))    # [128, RB, 16]

        # aug rows 0-3: column term (ones.T x parts); rows 4-7: row term
        aug_host = np.zeros((8, RPC + N), np.float32)
        for k in range(4):
            aug_host[k, RPC:] = parts32[k][colperm]
            aug_host[4 + k, RPC:] = AUGS
            aug_host[k, :RPC] = AUGS
            aug_host[4 + k, :RPC] = parts32[k][rows]

        in_maps.append({
            "ft8": np.ascontiguousarray(ft8_g[:, :, colperm]),
            "aug": aug_host.astype(BF),
            "rc": rc_host,
        })
    return in_maps, (s, cnt, ls, statm), (win_pad, win_w)


def _epilogue(out, out2, ctx):
    s, cnt, ls, statm = ctx
    hpos = out[:, 0].astype(np.float64)
    possum = out[:, 3].astype(np.float64)
    mind_p12 = out[:, 6].astype(np.float64)
    rowsum = (out[:, 8] + out[:, 9] + out[:, 10]
              + out[:, 11] + out[:, 12]).astype(np.float64)

    # out2: 0,1 = gen-3 max-of-h halves; 2,3 = gen-0 excl-window halves
    maxh = out2.max(1).astype(np.float64)
    hneg = np.minimum(np.sqrt(np.maximum(-2.0 * maxh + EPS, 0.0)), mind_p12)
    diag_dist = np.sqrt(EPS)
    pos_cnt = (cnt[ls] - 1).astype(np.float64)
    neg_cnt = (N - cnt[ls]).astype(np.float64)
    mean_pos = (possum - diag_dist) / np.maximum(pos_cnt, 1.0)
    mean_neg = (rowsum - possum) / np.maximum(neg_cnt, 1.0)
    final_margin = (BASE_MARGIN + ADAPTIVE_WEIGHT * (mean_neg - mean_pos)
                    + STAT_WEIGHT * statm)
    per_sample = np.maximum(hpos - hneg + final_margin, 0.0)
    valid = (pos_cnt > 0) & (neg_cnt > 0)
    n_valid = valid.sum()
    loss = per_sample[valid].sum() / max(n_valid, 1) if n_valid > 0 else 0.0
    return np.array(loss, dtype=np.float32)


def kernel(feats, labels):
    import sys
    if "/opt/trn_rl_repo" not in sys.path:
        sys.path.insert(0, "/opt/trn_rl_repo")
    from concourse.bass_utils import run_bass_kernel_spmd

    in_maps, ctx, (win_pad, win_w) = _prep_inputs(feats, labels)
    nc = _build(win_pad, win_w)
    trace = _maybe_enable_trace()
    import tempfile
    tmpdir = tempfile.mkdtemp(prefix="triplet_trace_") if trace else None
    res = run_bass_kernel_spmd(nc, in_maps, core_ids=list(range(NCORES)),
                               trace=bool(trace), tmpdir=tmpdir)
    global LAST_EXEC_NS, LAST_TRACE_DIR
    LAST_EXEC_NS = res.exec_time_ns
    LAST_TRACE_DIR = tmpdir

    # ---- host epilogue (O(N) numpy) ------------------------------------
    out = np.concatenate([
        np.asarray(res.results[c]["o_rows"]).transpose(1, 0, 2).reshape(RPC, 16)
        for c in range(NCORES)])
    out2 = np.concatenate([
        np.asarray(res.results[c]["o_tmr"]).transpose(1, 0, 2).reshape(RPC, 4)
        for c in range(NCORES)])
    import os
    if os.environ.get("V_DEBUG_OUT"):
        np.save(os.environ["V_DEBUG_OUT"], out)
    return _epilogue(out, out2, ctx)


if __name__ == "__main__":
    import jax
    key = jax.random.key(0)
    k1, k2 = jax.random.split(key)
    feats = np.asarray(jax.random.normal(k1, (N, D), dtype=np.float32))
    labels = np.asarray(jax.random.randint(k2, (N,), 0, NCLS, dtype=np.int32))
    out = kernel(feats=feats, labels=labels)
    print("kernel loss:", out)
